# revision 24
# baseline (speedup 1.0000x reference)
"""Trainium2 Bass kernel for KGAT-HAKE message passing (8 NeuronCores).

Degree-sorted node-row layout: every dst node owns one partition row of a
block; its incoming edges occupy consecutive columns of that row.  h-gather
is a broadcast, edge softmax is row-local, segment_sum is a free-dim
reduce -- no one-hot matmuls.  Blocks with equal edge-capacity L are
processed in groups with field-planar bf16 streams so every DVE op runs on
long packed runs.  Three SPMD launches; host re-gathers ego[src] between
launches (pure indexing).
"""
import numpy as np
import ml_dtypes
import concourse.bacc as bacc
import concourse.tile as tile
import concourse.mybir as mybir
from concourse import bass
from concourse.bass_utils import run_bass_kernel_spmd
from concourse.masks import make_identity

F32 = mybir.dt.float32
BF16 = mybir.dt.bfloat16
AF = mybir.ActivationFunctionType
OP = mybir.AluOpType
BF = ml_dtypes.bfloat16

N = 100000
E = 1000000
D = 64          # half width
TD = 2 * D      # 128
R = 40
NCORES = 8
SLOT = 128
NBLK = 98
PI = 3.1415926235897933
GAMMA = 12.0
EMB_RANGE = (GAMMA + 2.0) / D
SIN_SCALE = PI / (2.0 * EMB_RANGE)
WRAP_BOUND = PI / SIN_SCALE
WRAP_PERIOD = 2.0 * WRAP_BOUND
GMAX = 64      # max edge columns per processing group


# ----------------------------------------------------------------- host prep
def host_prep(inp):
    src = np.asarray(inp["src"]).astype(np.int64)
    dst = np.asarray(inp["dst"]).astype(np.int64)
    et = np.asarray(inp["etype"]).astype(np.int64)
    ent = np.asarray(inp["entity_embed"], dtype=np.float32)
    rel = np.asarray(inp["rel_embed"], dtype=np.float32)

    deg = np.bincount(dst, minlength=N)
    order = np.argsort(-deg, kind="stable")
    rank = np.empty(N, np.int64)
    rank[order] = np.arange(N)
    g = rank >> 7
    core_n = (g % NCORES).astype(np.int32)
    slot_n = (g // NCORES).astype(np.int32)
    m_n = (rank & 127).astype(np.int32)

    L = np.zeros(NBLK, np.int64)
    for s in range(NBLK):
        lo = 8 * s * SLOT
        if lo < N:
            L[s] = deg[order[lo]]
    off = np.zeros(NBLK + 1, np.int64)
    off[1:] = np.cumsum(L)
    ECOLS = int(off[-1])

    # groups of consecutive slots with equal L, capped at GMAX edge columns
    groups = []      # (s0, gb, ls)
    s = 0
    while s < NBLK:
        ls = int(L[s])
        if ls == 0:
            s += 1
            continue
        e_ = s
        while e_ < NBLK and int(L[e_]) == ls:
            e_ += 1
        cap = max(1, GMAX // ls)
        while s < e_:
            gb = min(cap, e_ - s)
            groups.append((s, gb, ls))
            s += gb

    eorder = np.argsort(dst, kind="stable")
    ds = dst[eorder]
    starts = np.searchsorted(ds, np.arange(N))
    l_e = np.empty(E, np.int64)
    l_e[eorder] = np.arange(E) - starts[ds]
    col_e = off[slot_n[dst]] + l_e
    core_e = core_n[dst]
    m_e = m_n[dst]

    nodetab = np.full((NCORES, NBLK, SLOT), -1, np.int64)
    nodetab[core_n, slot_n, m_n] = np.arange(N)

    pr = rel[:, :D]
    mr = np.abs(rel[:, D:2 * D])
    br = np.minimum(rel[:, 2 * D:], 1.0)
    br = np.maximum(br, -mr)
    c1 = mr + br
    c2 = 1.0 - br
    relf = np.concatenate([pr, c1, c2], axis=1).astype(BF).astype(np.float32)

    entb = ent.astype(BF).astype(np.float32)
    colslot = np.repeat(np.arange(NBLK), L)

    return dict(
        L=L, off=off, ECOLS=ECOLS, colslot=colslot, groups=groups,
        core_e=core_e, m_e=m_e, col_e=col_e, src=src, et=et,
        nodetab=nodetab, deg=deg, ent=ent, entb=entb, relf=relf,
        core_n=core_n, slot_n=slot_n, m_n=m_n,
    )


def _entc(lay, c, lo, hi):
    nt = lay["nodetab"][c]
    return lay["entb"][np.maximum(nt, 0), lo:hi] * (nt >= 0)[:, :, None]


def build_streams_A(lay):
    """phase stream [pt|pr planes], mod stream [mt|c1|br planes],
    msg stream [(d,l) per block] -- all group-planar bf16."""
    ECOLS, off, groups = lay["ECOLS"], lay["off"], lay["groups"]
    colslot = lay["colslot"]
    ph_l, md_l, ms_l, pc_l, eb_l = [], [], [], [], []
    for c in range(NCORES):
        entc = _entc(lay, c, 0, TD)        # [98, 128, 128]
        sel = lay["core_e"] == c
        em, ecol = lay["m_e"][sel], lay["col_e"][sel]
        t0 = np.zeros((SLOT, ECOLS, TD), np.float32)
        t0[em, ecol] = lay["entb"][lay["src"][sel]]
        rf = np.zeros((SLOT, ECOLS, 3 * D), np.float32)
        rf[:, :, 0:D] = -entc[colslot, :, 0:D].transpose(1, 0, 2)  # pad pr=-ph
        rf[em, ecol] = lay["relf"][lay["et"][sel]]

        nph = sum(2 * gb * ls * D for (_, gb, ls) in groups)
        nmd = sum(3 * gb * ls * D for (_, gb, ls) in groups)
        nms = sum(gb * ls * TD for (_, gb, ls) in groups)
        ph_s = np.empty((SLOT, nph), np.float32)
        md_s = np.empty((SLOT, nmd), np.float32)
        ms_s = np.empty((SLOT, nms), np.float32)
        po = mo = so = 0
        for (s0, gb, ls) in groups:
            c0, c1_ = int(off[s0]), int(off[s0] + gb * ls)
            w = gb * ls * D
            ph_s[:, po:po + w] = t0[:, c0:c1_, 0:D].reshape(SLOT, w)
            ph_s[:, po + w:po + 2 * w] = rf[:, c0:c1_, 0:D].reshape(SLOT, w)
            po += 2 * w
            md_s[:, mo:mo + w] = t0[:, c0:c1_, D:TD].reshape(SLOT, w)
            md_s[:, mo + w:mo + 2 * w] = rf[:, c0:c1_, D:TD].reshape(SLOT, w)
            md_s[:, mo + 2 * w:mo + 3 * w] = rf[:, c0:c1_, 2 * D:].reshape(
                SLOT, w)
            mo += 3 * w
            blk = t0[:, c0:c1_, :].reshape(SLOT, gb, ls, TD)
            ms_s[:, so:so + gb * ls * TD] = blk.transpose(0, 1, 3, 2).reshape(
                SLOT, gb * TD * ls)
            so += gb * ls * TD
        ph_l.append(ph_s.astype(BF))
        md_l.append(md_s.astype(BF))
        ms_l.append(ms_s.astype(BF))

        nt = lay["nodetab"][c]
        degc = lay["deg"][np.maximum(nt, 0)] * (nt >= 0)
        pc_l.append(np.ascontiguousarray(
            (lay["L"][:, None] - degc).astype(np.float32).T))
        eb_l.append(np.ascontiguousarray(
            entc.transpose(1, 0, 2).reshape(SLOT, NBLK * TD)).astype(BF))
    return ph_l, md_l, ms_l, pc_l, eb_l


def build_tstream(lay, tab, din):
    """tab [N, din] -> per-core group-planar (d, l)-layout bf16 stream."""
    off, groups = lay["off"], lay["groups"]
    tb = tab.astype(BF).astype(np.float32)
    ntot = sum(gb * ls * din for (_, gb, ls) in groups)
    out = []
    for c in range(NCORES):
        sel = lay["core_e"] == c
        buf = np.zeros((SLOT, lay["ECOLS"], din), np.float32)
        buf[lay["m_e"][sel], lay["col_e"][sel]] = tb[lay["src"][sel]]
        o = np.empty((SLOT, ntot), np.float32)
        so = 0
        for (s0, gb, ls) in groups:
            c0, c1_ = int(off[s0]), int(off[s0] + gb * ls)
            blk = buf[:, c0:c1_].reshape(SLOT, gb, ls, din)
            o[:, so:so + gb * ls * din] = blk.transpose(0, 1, 3, 2).reshape(
                SLOT, gb * din * ls)
            so += gb * ls * din
        out.append(o.astype(BF))
    return out


def node_table(lay, per_core, width):
    stk = np.stack([np.asarray(p, np.float32) for p in per_core])
    stk = stk.reshape(NCORES, SLOT, NBLK, width)
    return stk[lay["core_n"], lay["m_n"], lay["slot_n"]]


# ----------------------------------------------------------------- launch A
def build_A(lay, phase_w, mod_w):
    L, off, ECOLS, groups = lay["L"], lay["off"], lay["ECOLS"], lay["groups"]
    NPH = sum(2 * gb * ls * D for (_, gb, ls) in groups)
    NMD = sum(3 * gb * ls * D for (_, gb, ls) in groups)
    NMS = sum(gb * ls * TD for (_, gb, ls) in groups)
    nc = bacc.Bacc("TRN2", target_bir_lowering=False, debug=False,
                   num_devices=NCORES)
    d = {}
    d["ph_s"] = nc.dram_tensor("ph_s", [SLOT, NPH], BF16, kind="ExternalInput")
    d["md_s"] = nc.dram_tensor("md_s", [SLOT, NMD], BF16, kind="ExternalInput")
    d["ms_s"] = nc.dram_tensor("ms_s", [SLOT, NMS], BF16, kind="ExternalInput")
    d["entdb"] = nc.dram_tensor("entdb", [SLOT, NBLK * TD], BF16,
                                kind="ExternalInput")
    d["padc"] = nc.dram_tensor("padc", [SLOT, NBLK], F32, kind="ExternalInput")
    d["w1t"] = nc.dram_tensor("w1t", [TD, D], BF16, kind="ExternalInput")
    d["b1"] = nc.dram_tensor("b1", [1, D], BF16, kind="ExternalInput")
    d["w2t"] = nc.dram_tensor("w2t", [TD, D], BF16, kind="ExternalInput")
    d["b2"] = nc.dram_tensor("b2", [1, D], BF16, kind="ExternalInput")
    d["ex_o"] = nc.dram_tensor("ex_o", [SLOT, ECOLS], BF16,
                               kind="ExternalOutput")
    d["recden_o"] = nc.dram_tensor("recden_o", [SLOT, NBLK], F32,
                                   kind="ExternalOutput")
    d["ego_o"] = nc.dram_tensor("ego_o", [SLOT, NBLK * D], BF16,
                                kind="ExternalOutput")
    d["norm_o"] = nc.dram_tensor("norm_o", [SLOT, NBLK * D], F32,
                                 kind="ExternalOutput")

    with tile.TileContext(nc) as tc:
        with tc.tile_pool(name="pers", bufs=1) as pers:
            entdb_sb = pers.tile([SLOT, NBLK, TD], BF16)
            padc_sb = pers.tile([SLOT, NBLK], F32)
            ex_sb = pers.tile([SLOT, ECOLS], BF16)
            phs_sb = pers.tile([SLOT, ECOLS], BF16)
            mss_sb = pers.tile([SLOT, ECOLS], BF16)
            att_sb = pers.tile([SLOT, ECOLS], F32)
            den_sb = pers.tile([SLOT, NBLK], F32)
            recden_sb = pers.tile([SLOT, NBLK], F32)
            nh_sb = pers.tile([SLOT, NBLK, TD], BF16)
            o_sb = pers.tile([SLOT, NBLK, D], BF16)
            l1_sb = pers.tile([SLOT, NBLK, D], BF16)
            l2_sb = pers.tile([SLOT, NBLK, D], BF16)
            w1t_sb = pers.tile([TD, D], BF16)
            w2t_sb = pers.tile([TD, D], BF16)
            b1_sb = pers.tile([1, D], BF16)
            b2_sb = pers.tile([1, D], BF16)
            identb = pers.tile([SLOT, SLOT], BF16)
            ones_row = pers.tile([1, SLOT], BF16)

            nc.sync.dma_start(out=entdb_sb[:], in_=d["entdb"][:, :])
            nc.sync.dma_start(out=padc_sb[:], in_=d["padc"][:, :])
            nc.sync.dma_start(out=w1t_sb[:], in_=d["w1t"][:, :])
            nc.sync.dma_start(out=w2t_sb[:], in_=d["w2t"][:, :])
            nc.sync.dma_start(out=b1_sb[:], in_=d["b1"][:, :])
            nc.sync.dma_start(out=b2_sb[:], in_=d["b2"][:, :])
            make_identity(nc, identb[:])
            nc.vector.memset(ones_row[:], 1.0)
            nc.vector.memset(nh_sb[:], 0.0)
            nc.vector.memset(den_sb[:], 1.0)
            nc.vector.memset(ex_sb[:], 0.0)

            # -------- fused phase+mod score loop --------
            with tc.tile_pool(name="escore", bufs=2) as ep:
                po = mo = 0
                for (s0, gb, ls) in groups:
                    w = gb * ls * D
                    cols = slice(int(off[s0]), int(off[s0]) + gb * ls)
                    pht = ep.tile([SLOT, 2 * GMAX * D], BF16, tag="pht")
                    mdt = ep.tile([SLOT, 3 * GMAX * D], BF16, tag="mdt")
                    b = ep.tile([SLOT, GMAX * D], BF16, tag="b")
                    nc.sync.dma_start(out=pht[:, :2 * w],
                                      in_=d["ph_s"][:, po:po + 2 * w])
                    nc.sync.dma_start(out=mdt[:, :3 * w],
                                      in_=d["md_s"][:, mo:mo + 3 * w])
                    po += 2 * w
                    mo += 3 * w
                    ptf = pht[:, 0:w]
                    prf = pht[:, w:2 * w]
                    mtf = mdt[:, 0:w]
                    c1f = mdt[:, w:2 * w]
                    brf = mdt[:, 2 * w:3 * w]
                    bf = b[:, :w]
                    pr4 = prf.rearrange("p (b l dd) -> p b l dd", b=gb, dd=D)
                    c14 = c1f.rearrange("p (b l dd) -> p b l dd", b=gb, dd=D)
                    b4 = bf.rearrange("p (b l dd) -> p b l dd", b=gb, dd=D)
                    ph_b = entdb_sb[:, s0:s0 + gb, 0:D].unsqueeze(2) \
                        .to_broadcast([SLOT, gb, ls, D])
                    mh_b = entdb_sb[:, s0:s0 + gb, D:TD].unsqueeze(2) \
                        .to_broadcast([SLOT, gb, ls, D])
                    # phase: s1 = (pr - pt) + ph; sin; sum |.|  (no wrap:
                    # |arg| <= 1.5pi and the sin table stays close enough)
                    nc.vector.tensor_tensor(out=prf, in0=prf, in1=ptf,
                                            op=OP.subtract)
                    nc.vector.tensor_tensor(out=pr4, in0=pr4, in1=ph_b,
                                            op=OP.add)
                    nc.scalar.activation(out=bf, in_=prf, func=AF.Sin,
                                         scale=SIN_SCALE)
                    with nc.allow_low_precision(reason="bf16 att scores"):
                        nc.vector.tensor_reduce(
                            out=phs_sb[:, cols].rearrange("p (b l) -> p b l",
                                                          b=gb),
                            in_=b4, axis=mybir.AxisListType.X, op=OP.add,
                            apply_absolute_value=True)
                    # mod: v = mh*c1 - mt*c2 ; mss = sum v^2
                    nc.gpsimd.tensor_tensor(out=mtf, in0=mtf, in1=brf,
                                            op=OP.mult)
                    nc.vector.tensor_tensor(out=c14, in0=c14, in1=mh_b,
                                            op=OP.mult)
                    nc.gpsimd.tensor_tensor(out=c1f, in0=c1f, in1=mtf,
                                            op=OP.subtract)
                    nc.scalar.activation(out=bf, in_=c1f, func=AF.Square)
                    with nc.allow_low_precision(reason="bf16 att scores"):
                        nc.vector.tensor_reduce(
                            out=mss_sb[:, cols].rearrange("p (b l) -> p b l",
                                                          b=gb),
                            in_=b4, axis=mybir.AxisListType.X, op=OP.add)

            # -------- att + softmax numerator (batched) --------
            nc.scalar.activation(out=mss_sb[:], in_=mss_sb[:], func=AF.Sqrt,
                                 scale=float(mod_w * mod_w))
            nc.vector.scalar_tensor_tensor(out=att_sb[:], in0=phs_sb[:],
                                           scalar=float(phase_w),
                                           in1=mss_sb[:], op0=OP.mult,
                                           op1=OP.add)
            nc.scalar.activation(out=ex_sb[:], in_=att_sb[:], func=AF.Exp)
            nc.vector.memset(recden_sb[:], 1.0)
            nc.sync.dma_start(out=d["ex_o"][:, :], in_=ex_sb[:])

            # -------- msg + Nh + dense, interleaved per group --------
            def dense_blk(s, dp, pp):
                x1tp = pp.tile([SLOT, SLOT], BF16, tag="x1tp")
                x2tp = pp.tile([SLOT, SLOT], BF16, tag="x2tp")
                nc.tensor.transpose(out=x1tp[:], in_=x1_sb[:, s, :],
                                    identity=identb[:])
                nc.tensor.transpose(out=x2tp[:], in_=x2_sb[:, s, :],
                                    identity=identb[:])
                x1t = dp.tile([SLOT, SLOT], BF16, tag="x1t")
                x2t = dp.tile([SLOT, SLOT], BF16, tag="x2t")
                nc.scalar.copy(out=x1t[:], in_=x1tp[:])
                nc.scalar.copy(out=x2t[:], in_=x2tp[:])
                o1p = pp.tile([SLOT, D], F32, tag="o1p")
                o2p = pp.tile([SLOT, D], F32, tag="o2p")
                nc.tensor.matmul(out=o1p[:], lhsT=x1t[:], rhs=w1t_sb[:],
                                 start=True, stop=False)
                nc.tensor.matmul(out=o1p[:], lhsT=ones_row[:], rhs=b1_sb[:],
                                 start=False, stop=True)
                nc.tensor.matmul(out=o2p[:], lhsT=x2t[:], rhs=w2t_sb[:],
                                 start=True, stop=False)
                nc.tensor.matmul(out=o2p[:], lhsT=ones_row[:], rhs=b2_sb[:],
                                 start=False, stop=True)
                nc.scalar.activation(out=l1_sb[:, s, :], in_=o1p[:],
                                     func=AF.Lrelu, alpha=0.01)
                nc.scalar.activation(out=l2_sb[:, s, :], in_=o2p[:],
                                     func=AF.Lrelu, alpha=0.01)

            with tc.tile_pool(name="xd", bufs=1) as xdp, \
                 tc.tile_pool(name="emsg", bufs=2) as mp, \
                 tc.tile_pool(name="dense", bufs=3) as dp, \
                 tc.tile_pool(name="dpsum", bufs=2, space="PSUM") as pp:
                x1_sb = xdp.tile([SLOT, NBLK, TD], BF16)
                x2_sb = xdp.tile([SLOT, NBLK, TD], BF16)
                so = 0
                cur = 0
                for (s0, gb, ls) in groups:
                    wm = gb * ls * TD
                    cols = slice(int(off[s0]), int(off[s0]) + gb * ls)
                    blks = slice(s0, s0 + gb)
                    nc.vector.tensor_reduce(
                        out=den_sb[:, blks],
                        in_=ex_sb[:, cols].rearrange("p (b l) -> p b l",
                                                     b=gb),
                        axis=mybir.AxisListType.X, op=OP.add)
                    nc.vector.tensor_tensor(out=den_sb[:, blks],
                                            in0=den_sb[:, blks],
                                            in1=padc_sb[:, blks],
                                            op=OP.subtract)
                    nc.vector.tensor_scalar_max(out=den_sb[:, blks],
                                                in0=den_sb[:, blks],
                                                scalar1=1e-30)
                    nc.vector.reciprocal(out=recden_sb[:, blks],
                                         in_=den_sb[:, blks])
                    ms = mp.tile([SLOT, GMAX * TD], BF16, tag="ms")
                    nc.sync.dma_start(out=ms[:, :wm],
                                      in_=d["ms_s"][:, so:so + wm])
                    so += wm
                    ms4 = ms[:, :wm].rearrange("p (b dd l) -> p b dd l",
                                               b=gb, dd=TD)
                    ex4 = ex_sb[:, cols].rearrange("p (b l) -> p b l", b=gb) \
                        .unsqueeze(2).to_broadcast([SLOT, gb, TD, ls])
                    nc.vector.tensor_tensor(out=ms4, in0=ms4, in1=ex4,
                                            op=OP.mult)
                    with nc.allow_low_precision(reason="bf16 Nh"):
                        nc.vector.tensor_reduce(out=nh_sb[:, blks, :],
                                                in_=ms4,
                                                axis=mybir.AxisListType.X,
                                                op=OP.add)
                    rdb = recden_sb[:, blks].unsqueeze(2).to_broadcast(
                        [SLOT, gb, TD])
                    nc.gpsimd.tensor_tensor(out=x2_sb[:, blks, :],
                                            in0=nh_sb[:, blks, :], in1=rdb,
                                            op=OP.mult)
                    nc.gpsimd.tensor_tensor(out=x1_sb[:, blks, :],
                                            in0=x2_sb[:, blks, :],
                                            in1=entdb_sb[:, blks, :],
                                            op=OP.add)
                    nc.gpsimd.tensor_tensor(out=x2_sb[:, blks, :],
                                            in0=x2_sb[:, blks, :],
                                            in1=entdb_sb[:, blks, :],
                                            op=OP.mult)
                    for s in range(cur, s0 + gb):
                        dense_blk(s, dp, pp)
                    cur = s0 + gb
                # tail: blocks not covered by any group (all-zero-degree)
                for s in range(cur, NBLK):
                    nc.vector.tensor_tensor(out=x2_sb[:, s, :],
                                            in0=nh_sb[:, s, :],
                                            in1=recden_sb[:, s:s + 1]
                                            .to_broadcast([SLOT, TD]),
                                            op=OP.mult)
                    nc.vector.tensor_tensor(out=x1_sb[:, s, :],
                                            in0=x2_sb[:, s, :],
                                            in1=entdb_sb[:, s, :], op=OP.add)
                    nc.vector.tensor_tensor(out=x2_sb[:, s, :],
                                            in0=x2_sb[:, s, :],
                                            in1=entdb_sb[:, s, :],
                                            op=OP.mult)
                    dense_blk(s, dp, pp)
            nc.sync.dma_start(out=d["recden_o"][:, :], in_=recden_sb[:])
            nc.vector.tensor_tensor(out=o_sb[:], in0=l1_sb[:], in1=l2_sb[:],
                                    op=OP.add)
            nc.sync.dma_start(
                out=d["ego_o"][:, :],
                in_=o_sb[:].rearrange("p b dd -> p (b dd)"))
            # -------- norm --------
            with tc.tile_pool(name="npool", bufs=1) as np_:
                sq_sb = np_.tile([SLOT, NBLK, D], BF16)
                nrm_sb = np_.tile([SLOT, NBLK, D], F32)
                nc.scalar.activation(out=sq_sb[:], in_=o_sb[:],
                                     func=AF.Square)
                ss = np_.tile([SLOT, NBLK], F32)
                nc.vector.tensor_reduce(out=ss[:], in_=sq_sb[:],
                                        axis=mybir.AxisListType.X, op=OP.add)
                nc.scalar.activation(out=ss[:], in_=ss[:], func=AF.Sqrt)
                nc.vector.tensor_scalar_max(out=ss[:], in0=ss[:],
                                            scalar1=1e-12)
                rs = np_.tile([SLOT, NBLK], F32)
                nc.vector.reciprocal(out=rs[:], in_=ss[:])
                nc.vector.tensor_tensor(
                    out=nrm_sb[:], in0=o_sb[:],
                    in1=rs[:].unsqueeze(2).to_broadcast([SLOT, NBLK, D]),
                    op=OP.mult)
                nc.sync.dma_start(
                    out=d["norm_o"][:, :],
                    in_=nrm_sb[:].rearrange("p b dd -> p (b dd)"))

    nc.compile()
    return nc


# ----------------------------------------------------------------- launch B/C
def build_BC(lay, din, dout):
    """Dense phase packs x1||x2 (and PK blocks) into one transpose+matmul
    with a block-diagonal weight tile."""
    L, off, ECOLS, groups = lay["L"], lay["off"], lay["ECOLS"], lay["groups"]
    NT = sum(gb * ls * din for (_, gb, ls) in groups)
    PK = SLOT // (2 * din)          # blocks per transpose (B:1, C:2)
    WC = 2 * PK * dout              # output cols per packed matmul
    nc = bacc.Bacc("TRN2", target_bir_lowering=False, debug=False,
                   num_devices=NCORES)
    d = {}
    d["t"] = nc.dram_tensor("t", [SLOT, NT], BF16, kind="ExternalInput")
    d["ex_i"] = nc.dram_tensor("ex_i", [SLOT, ECOLS], BF16,
                               kind="ExternalInput")
    d["recden_i"] = nc.dram_tensor("recden_i", [SLOT, NBLK], F32,
                                   kind="ExternalInput")
    d["egod"] = nc.dram_tensor("egod", [SLOT, NBLK * din], BF16,
                               kind="ExternalInput")
    d["wbd"] = nc.dram_tensor("wbd", [SLOT, WC], BF16, kind="ExternalInput")
    d["bbd"] = nc.dram_tensor("bbd", [1, WC], BF16, kind="ExternalInput")
    d["ego_o"] = nc.dram_tensor("ego_o", [SLOT, NBLK * dout], BF16,
                                kind="ExternalOutput")
    d["norm_o"] = nc.dram_tensor("norm_o", [SLOT, NBLK * dout], F32,
                                 kind="ExternalOutput")

    with tile.TileContext(nc) as tc:
        with tc.tile_pool(name="pers", bufs=1) as pers:
            ex_sb = pers.tile([SLOT, ECOLS], BF16)
            recden_sb = pers.tile([SLOT, NBLK], F32)
            egod_sb = pers.tile([SLOT, NBLK, din], BF16)
            nh_sb = pers.tile([SLOT, NBLK, din], BF16)
            xp_sb = pers.tile([SLOT, NBLK, 2, din], BF16)
            lp_sb = pers.tile([SLOT, NBLK, 2, dout], BF16)
            o_sb = pers.tile([SLOT, NBLK, dout], BF16)
            wbd_sb = pers.tile([SLOT, WC], BF16)
            bbd_sb = pers.tile([1, WC], BF16)
            identb = pers.tile([SLOT, SLOT], BF16)
            ones_row = pers.tile([1, SLOT], BF16)

            nc.sync.dma_start(out=ex_sb[:], in_=d["ex_i"][:, :])
            nc.sync.dma_start(out=recden_sb[:], in_=d["recden_i"][:, :])
            nc.sync.dma_start(out=egod_sb[:], in_=d["egod"][:, :])
            nc.sync.dma_start(out=wbd_sb[:], in_=d["wbd"][:, :])
            nc.sync.dma_start(out=bbd_sb[:], in_=d["bbd"][:, :])
            make_identity(nc, identb[:])
            nc.vector.memset(ones_row[:], 1.0)
            nc.vector.memset(nh_sb[:], 0.0)

            def dense_chunk(s, dp, pp):
                xtp = pp.tile([SLOT, SLOT], BF16, tag="xtp")
                nc.tensor.transpose(out=xtp[:], in_=xp_sb[:, s:s + PK, :, :],
                                    identity=identb[:])
                xt = dp.tile([SLOT, SLOT], BF16, tag="xt")
                nc.scalar.copy(out=xt[:], in_=xtp[:])
                op_ = pp.tile([SLOT, WC], F32, tag="op")
                nc.tensor.matmul(out=op_[:], lhsT=xt[:], rhs=wbd_sb[:],
                                 start=True, stop=False)
                nc.tensor.matmul(out=op_[:], lhsT=ones_row[:], rhs=bbd_sb[:],
                                 start=False, stop=True)
                nc.scalar.activation(out=lp_sb[:, s:s + PK, :, :],
                                     in_=op_[:], func=AF.Lrelu, alpha=0.01)

            def x_ops(blks, gb):
                rdb = recden_sb[:, blks].unsqueeze(2).to_broadcast(
                    [SLOT, gb, din])
                nhr = xp_sb[:, blks, 1, :]
                nc.gpsimd.tensor_tensor(out=nhr, in0=nh_sb[:, blks, :],
                                        in1=rdb, op=OP.mult)
                nc.gpsimd.tensor_tensor(out=xp_sb[:, blks, 0, :], in0=nhr,
                                        in1=egod_sb[:, blks, :], op=OP.add)
                nc.gpsimd.tensor_tensor(out=xp_sb[:, blks, 1, :], in0=nhr,
                                        in1=egod_sb[:, blks, :], op=OP.mult)

            with tc.tile_pool(name="emsg", bufs=2) as mp, \
                 tc.tile_pool(name="dense", bufs=3) as dp, \
                 tc.tile_pool(name="dpsum", bufs=2, space="PSUM") as pp:
                so = 0
                cur = 0
                for (s0, gb, ls) in groups:
                    wm = gb * ls * din
                    cols = slice(int(off[s0]), int(off[s0]) + gb * ls)
                    blks = slice(s0, s0 + gb)
                    ms = mp.tile([SLOT, GMAX * din], BF16, tag="ms")
                    nc.sync.dma_start(out=ms[:, :wm],
                                      in_=d["t"][:, so:so + wm])
                    so += wm
                    ms4 = ms[:, :wm].rearrange("p (b dd l) -> p b dd l",
                                               b=gb, dd=din)
                    ex4 = ex_sb[:, cols].rearrange("p (b l) -> p b l", b=gb) \
                        .unsqueeze(2).to_broadcast([SLOT, gb, din, ls])
                    nc.vector.tensor_tensor(out=ms4, in0=ms4, in1=ex4,
                                            op=OP.mult)
                    with nc.allow_low_precision(reason="bf16 Nh"):
                        nc.vector.tensor_reduce(out=nh_sb[:, blks, :],
                                                in_=ms4,
                                                axis=mybir.AxisListType.X,
                                                op=OP.add)
                    x_ops(blks, gb)
                    while cur + PK <= s0 + gb:
                        dense_chunk(cur, dp, pp)
                        cur += PK
                if cur < NBLK:
                    x_ops(slice(cur, NBLK), NBLK - cur)
                    while cur < NBLK:
                        dense_chunk(cur, dp, pp)
                        cur += PK
            nc.vector.tensor_tensor(out=o_sb[:], in0=lp_sb[:, :, 0, :],
                                    in1=lp_sb[:, :, 1, :], op=OP.add)
            nc.sync.dma_start(
                out=d["ego_o"][:, :],
                in_=o_sb[:].rearrange("p b dd -> p (b dd)"))
            with tc.tile_pool(name="npool", bufs=1) as np_:
                sq_sb = np_.tile([SLOT, NBLK, dout], BF16)
                nrm_sb = np_.tile([SLOT, NBLK, dout], F32)
                nc.scalar.activation(out=sq_sb[:], in_=o_sb[:],
                                     func=AF.Square)
                ss = np_.tile([SLOT, NBLK], F32)
                nc.vector.tensor_reduce(out=ss[:], in_=sq_sb[:],
                                        axis=mybir.AxisListType.X, op=OP.add)
                nc.scalar.activation(out=ss[:], in_=ss[:], func=AF.Sqrt)
                nc.vector.tensor_scalar_max(out=ss[:], in0=ss[:],
                                            scalar1=1e-12)
                rs = np_.tile([SLOT, NBLK], F32)
                nc.vector.reciprocal(out=rs[:], in_=ss[:])
                nc.vector.tensor_tensor(
                    out=nrm_sb[:], in0=o_sb[:],
                    in1=rs[:].unsqueeze(2).to_broadcast([SLOT, NBLK, dout]),
                    op=OP.mult)
                nc.sync.dma_start(
                    out=d["norm_o"][:, :],
                    in_=nrm_sb[:].rearrange("p b dd -> p (b dd)"))

    nc.compile()
    return nc


def make_wbd(W1, b1, W2, b2, din, dout):
    PK = SLOT // (2 * din)
    WC = 2 * PK * dout
    wbd = np.zeros((SLOT, WC), np.float32)
    bbd = np.zeros((1, WC), np.float32)
    for k in range(PK):
        r0 = k * 2 * din
        c0 = k * 2 * dout
        wbd[r0:r0 + din, c0:c0 + dout] = np.asarray(W1, np.float32).T
        wbd[r0 + din:r0 + 2 * din, c0 + dout:c0 + 2 * dout] = \
            np.asarray(W2, np.float32).T
        bbd[0, c0:c0 + dout] = np.asarray(b1, np.float32)
        bbd[0, c0 + dout:c0 + 2 * dout] = np.asarray(b2, np.float32)
    return wbd.astype(BF), bbd.astype(BF)


# ----------------------------------------------------------------- driver
def run(inp, trace=False, verbose=True):
    import time
    t0c = time.time()
    lay = host_prep(inp)
    if verbose:
        print(f"host_prep: ECOLS={lay['ECOLS']} groups={len(lay['groups'])} "
              f"({time.time()-t0c:.1f}s)")
    phase_w = float(np.asarray(inp["phase_w"]).reshape(-1)[0])
    mod_w = float(np.asarray(inp["mod_w"]).reshape(-1)[0])

    ph_l, md_l, ms_l, pc_l, eb_l = build_streams_A(lay)
    if verbose:
        print(f"streams built ({time.time()-t0c:.1f}s)")

    exec_ns = 0
    t0c = time.time()
    ncA = build_A(lay, phase_w, mod_w)
    if verbose:
        print(f"A compiled in {time.time()-t0c:.1f}s")
    in_maps = []
    for c in range(NCORES):
        in_maps.append(dict(
            ph_s=ph_l[c], md_s=md_l[c], ms_s=ms_l[c], entdb=eb_l[c],
            padc=pc_l[c],
            w1t=np.ascontiguousarray(np.asarray(inp["W1_0"]).T).astype(BF),
            b1=np.asarray(inp["b1_0"]).reshape(1, -1).astype(BF),
            w2t=np.ascontiguousarray(np.asarray(inp["W2_0"]).T).astype(BF),
            b2=np.asarray(inp["b2_0"]).reshape(1, -1).astype(BF),
        ))
    t0c = time.time()
    resA = run_bass_kernel_spmd(ncA, in_maps, core_ids=list(range(NCORES)),
                                trace=trace)
    if verbose:
        print(f"A ran in {time.time()-t0c:.1f}s exec_ns={resA.exec_time_ns}")
    if resA.exec_time_ns:
        exec_ns += resA.exec_time_ns

    ego1 = node_table(lay, [r["ego_o"] for r in resA.results], D)
    norm1 = node_table(lay, [r["norm_o"] for r in resA.results], D)
    ex_pc = [np.asarray(r["ex_o"]) for r in resA.results]
    recden_pc = [np.asarray(r["recden_o"]) for r in resA.results]

    t1s = build_tstream(lay, ego1, D)
    t0c = time.time()
    ncB = build_BC(lay, D, 32)
    if verbose:
        print(f"B compiled in {time.time()-t0c:.1f}s")
    wbdB, bbdB = make_wbd(inp["W1_1"], inp["b1_1"], inp["W2_1"],
                          inp["b2_1"], D, 32)
    in_maps = []
    for c in range(NCORES):
        in_maps.append(dict(
            t=t1s[c], ex_i=ex_pc[c], recden_i=recden_pc[c],
            egod=np.asarray(resA.results[c]["ego_o"]).astype(BF),
            wbd=wbdB, bbd=bbdB,
        ))
    t0c = time.time()
    resB = run_bass_kernel_spmd(ncB, in_maps, core_ids=list(range(NCORES)),
                                trace=trace)
    if verbose:
        print(f"B ran in {time.time()-t0c:.1f}s exec_ns={resB.exec_time_ns}")
    if resB.exec_time_ns:
        exec_ns += resB.exec_time_ns
    ego2 = node_table(lay, [r["ego_o"] for r in resB.results], 32)
    norm2 = node_table(lay, [r["norm_o"] for r in resB.results], 32)

    t2s = build_tstream(lay, ego2, 32)
    t0c = time.time()
    ncC = build_BC(lay, 32, 16)
    if verbose:
        print(f"C compiled in {time.time()-t0c:.1f}s")
    wbdC, bbdC = make_wbd(inp["W1_2"], inp["b1_2"], inp["W1_2b"],
                          inp["b2_2"], 32, 16)
    in_maps = []
    for c in range(NCORES):
        in_maps.append(dict(
            t=t2s[c], ex_i=ex_pc[c], recden_i=recden_pc[c],
            egod=np.asarray(resB.results[c]["ego_o"]).astype(BF),
            wbd=wbdC, bbd=bbdC,
        ))
    t0c = time.time()
    resC = run_bass_kernel_spmd(ncC, in_maps, core_ids=list(range(NCORES)),
                                trace=trace)
    if verbose:
        print(f"C ran in {time.time()-t0c:.1f}s exec_ns={resC.exec_time_ns}")
    if resC.exec_time_ns:
        exec_ns += resC.exec_time_ns
    norm3 = node_table(lay, [r["norm_o"] for r in resC.results], 16)

    ent = np.asarray(inp["entity_embed"], dtype=np.float32)
    out = np.concatenate([ent, norm1, norm2, norm3], axis=1)
    return out, exec_ns


# ----------------------------------------------------------------- entry
TRACE = False
LAST_EXEC_NS = None


def _install_ntff_hook():
    import sys, types
    if "antenv.axon_hooks" in sys.modules:
        return True
    try:
        mod = types.ModuleType("antenv.axon_hooks")
        mod._hook = None
        mod.set_axon_ntff_profile_hook = lambda h: setattr(mod, "_hook", h)
        mod.get_axon_ntff_profile_hook = lambda: mod._hook
        import antenv
        sys.modules["antenv.axon_hooks"] = mod
        antenv.axon_hooks = mod
        from trn_agent_boot.trn_boot import _ntff_profile_via_ctypes
        h = _ntff_profile_via_ctypes("/opt/axon/libaxon_pjrt.so")
        if h is None:
            return False
        mod._hook = h
        return True
    except Exception:
        return False


def kernel(**inputs):
    global LAST_EXEC_NS
    trace = TRACE and _install_ntff_hook()
    out, exec_ns = run(inputs, trace=trace, verbose=False)
    LAST_EXEC_NS = exec_ns
    return out


# revision 26
# speedup vs baseline: 1.0802x; 1.0802x over previous
"""Trainium2 Bass kernel for KGAT-HAKE message passing (8 NeuronCores).

Degree-sorted node-row layout: every dst node owns one partition row of a
block; its incoming edges occupy consecutive columns of that row.  h-gather
is a broadcast, edge softmax is row-local, segment_sum is a free-dim
reduce -- no one-hot matmuls.  Blocks with equal edge-capacity L are
processed in groups with field-planar bf16 streams so every DVE op runs on
long packed runs.  Three SPMD launches; host re-gathers ego[src] between
launches (pure indexing).
"""
import numpy as np
import ml_dtypes
import concourse.bacc as bacc
import concourse.tile as tile
import concourse.mybir as mybir
from concourse import bass
from concourse.bass_utils import run_bass_kernel_spmd
from concourse.masks import make_identity

F32 = mybir.dt.float32
BF16 = mybir.dt.bfloat16
AF = mybir.ActivationFunctionType
OP = mybir.AluOpType
BF = ml_dtypes.bfloat16

N = 100000
E = 1000000
D = 64          # half width
TD = 2 * D      # 128
R = 40
NCORES = 8
SLOT = 128
NBLK = 98
PI = 3.1415926235897933
GAMMA = 12.0
EMB_RANGE = (GAMMA + 2.0) / D
SIN_SCALE = PI / (2.0 * EMB_RANGE)
WRAP_BOUND = PI / SIN_SCALE
WRAP_PERIOD = 2.0 * WRAP_BOUND
GMAX = 64      # max edge columns per processing group


# ----------------------------------------------------------------- host prep
def host_prep(inp):
    src = np.asarray(inp["src"]).astype(np.int64)
    dst = np.asarray(inp["dst"]).astype(np.int64)
    et = np.asarray(inp["etype"]).astype(np.int64)
    ent = np.asarray(inp["entity_embed"], dtype=np.float32)
    rel = np.asarray(inp["rel_embed"], dtype=np.float32)

    deg = np.bincount(dst, minlength=N)
    order = np.argsort(-deg, kind="stable")
    rank = np.empty(N, np.int64)
    rank[order] = np.arange(N)
    g = rank >> 7
    core_n = (g % NCORES).astype(np.int32)
    slot_n = (g // NCORES).astype(np.int32)
    m_n = (rank & 127).astype(np.int32)

    L = np.zeros(NBLK, np.int64)
    for s in range(NBLK):
        lo = 8 * s * SLOT
        if lo < N:
            L[s] = deg[order[lo]]
    off = np.zeros(NBLK + 1, np.int64)
    off[1:] = np.cumsum(L)
    ECOLS = int(off[-1])

    # groups of consecutive slots with equal L, capped at GMAX edge columns
    groups = []      # (s0, gb, ls)
    s = 0
    while s < NBLK:
        ls = int(L[s])
        if ls == 0:
            s += 1
            continue
        e_ = s
        while e_ < NBLK and int(L[e_]) == ls:
            e_ += 1
        cap = max(1, GMAX // ls)
        while s < e_:
            gb = min(cap, e_ - s)
            groups.append((s, gb, ls))
            s += gb

    eorder = np.argsort(dst, kind="stable")
    ds = dst[eorder]
    starts = np.searchsorted(ds, np.arange(N))
    l_e = np.empty(E, np.int64)
    l_e[eorder] = np.arange(E) - starts[ds]
    col_e = off[slot_n[dst]] + l_e
    core_e = core_n[dst]
    m_e = m_n[dst]

    nodetab = np.full((NCORES, NBLK, SLOT), -1, np.int64)
    nodetab[core_n, slot_n, m_n] = np.arange(N)

    pr = rel[:, :D]
    mr = np.abs(rel[:, D:2 * D])
    br = np.minimum(rel[:, 2 * D:], 1.0)
    br = np.maximum(br, -mr)
    c1 = mr + br
    c2 = 1.0 - br
    relf = np.concatenate([pr, c1, c2], axis=1).astype(BF).astype(np.float32)

    entb = ent.astype(BF).astype(np.float32)
    colslot = np.repeat(np.arange(NBLK), L)

    return dict(
        L=L, off=off, ECOLS=ECOLS, colslot=colslot, groups=groups,
        core_e=core_e, m_e=m_e, col_e=col_e, src=src, et=et,
        nodetab=nodetab, deg=deg, ent=ent, entb=entb, relf=relf,
        core_n=core_n, slot_n=slot_n, m_n=m_n,
    )


def _entc(lay, c, lo, hi):
    nt = lay["nodetab"][c]
    return lay["entb"][np.maximum(nt, 0), lo:hi] * (nt >= 0)[:, :, None]


def build_streams_A(lay):
    """phase stream [pt|pr planes], mod stream [mt|c1|br planes],
    msg stream [(d,l) per block] -- all group-planar bf16."""
    ECOLS, off, groups = lay["ECOLS"], lay["off"], lay["groups"]
    colslot = lay["colslot"]
    ph_l, md_l, ms_l, pc_l, eb_l = [], [], [], [], []
    for c in range(NCORES):
        entc = _entc(lay, c, 0, TD)        # [98, 128, 128]
        sel = lay["core_e"] == c
        em, ecol = lay["m_e"][sel], lay["col_e"][sel]
        t0 = np.zeros((SLOT, ECOLS, TD), np.float32)
        t0[em, ecol] = lay["entb"][lay["src"][sel]]
        rf = np.zeros((SLOT, ECOLS, 3 * D), np.float32)
        rf[:, :, 0:D] = -entc[colslot, :, 0:D].transpose(1, 0, 2)  # pad pr=-ph
        rf[em, ecol] = lay["relf"][lay["et"][sel]]

        nph = sum(2 * gb * ls * D for (_, gb, ls) in groups)
        nmd = sum(3 * gb * ls * D for (_, gb, ls) in groups)
        nms = sum(gb * ls * TD for (_, gb, ls) in groups)
        ph_s = np.empty((SLOT, nph), np.float32)
        md_s = np.empty((SLOT, nmd), np.float32)
        ms_s = np.empty((SLOT, nms), np.float32)
        po = mo = so = 0
        for (s0, gb, ls) in groups:
            c0, c1_ = int(off[s0]), int(off[s0] + gb * ls)
            w = gb * ls * D
            ph_s[:, po:po + w] = t0[:, c0:c1_, 0:D].reshape(SLOT, w)
            ph_s[:, po + w:po + 2 * w] = rf[:, c0:c1_, 0:D].reshape(SLOT, w)
            po += 2 * w
            md_s[:, mo:mo + w] = t0[:, c0:c1_, D:TD].reshape(SLOT, w)
            md_s[:, mo + w:mo + 2 * w] = rf[:, c0:c1_, D:TD].reshape(SLOT, w)
            md_s[:, mo + 2 * w:mo + 3 * w] = rf[:, c0:c1_, 2 * D:].reshape(
                SLOT, w)
            mo += 3 * w
            blk = t0[:, c0:c1_, :].reshape(SLOT, gb, ls, TD)
            ms_s[:, so:so + gb * ls * TD] = blk.transpose(0, 1, 3, 2).reshape(
                SLOT, gb * TD * ls)
            so += gb * ls * TD
        ph_l.append(ph_s.astype(BF))
        md_l.append(md_s.astype(BF))
        ms_l.append(ms_s.astype(BF))

        nt = lay["nodetab"][c]
        degc = lay["deg"][np.maximum(nt, 0)] * (nt >= 0)
        pc_l.append(np.ascontiguousarray(
            (lay["L"][:, None] - degc).astype(np.float32).T))
        eb_l.append(np.ascontiguousarray(
            entc.transpose(1, 0, 2).reshape(SLOT, NBLK * TD)).astype(BF))
    return ph_l, md_l, ms_l, pc_l, eb_l


def build_tstream(lay, tab, din):
    """tab [N, din] -> per-core group-planar (d, l)-layout bf16 stream."""
    off, groups = lay["off"], lay["groups"]
    tb = tab.astype(BF).astype(np.float32)
    ntot = sum(gb * ls * din for (_, gb, ls) in groups)
    out = []
    for c in range(NCORES):
        sel = lay["core_e"] == c
        buf = np.zeros((SLOT, lay["ECOLS"], din), np.float32)
        buf[lay["m_e"][sel], lay["col_e"][sel]] = tb[lay["src"][sel]]
        o = np.empty((SLOT, ntot), np.float32)
        so = 0
        for (s0, gb, ls) in groups:
            c0, c1_ = int(off[s0]), int(off[s0] + gb * ls)
            blk = buf[:, c0:c1_].reshape(SLOT, gb, ls, din)
            o[:, so:so + gb * ls * din] = blk.transpose(0, 1, 3, 2).reshape(
                SLOT, gb * din * ls)
            so += gb * ls * din
        out.append(o.astype(BF))
    return out


def node_table(lay, per_core, width):
    stk = np.stack([np.asarray(p, np.float32) for p in per_core])
    stk = stk.reshape(NCORES, SLOT, NBLK, width)
    return stk[lay["core_n"], lay["m_n"], lay["slot_n"]]


# ----------------------------------------------------------------- launch A
def build_A(lay, phase_w, mod_w):
    L, off, ECOLS, groups = lay["L"], lay["off"], lay["ECOLS"], lay["groups"]
    NPH = sum(2 * gb * ls * D for (_, gb, ls) in groups)
    NMD = sum(3 * gb * ls * D for (_, gb, ls) in groups)
    NMS = sum(gb * ls * TD for (_, gb, ls) in groups)
    nc = bacc.Bacc("TRN2", target_bir_lowering=False, debug=False,
                   num_devices=NCORES)
    d = {}
    d["ph_s"] = nc.dram_tensor("ph_s", [SLOT, NPH], BF16, kind="ExternalInput")
    d["md_s"] = nc.dram_tensor("md_s", [SLOT, NMD], BF16, kind="ExternalInput")
    d["ms_s"] = nc.dram_tensor("ms_s", [SLOT, NMS], BF16, kind="ExternalInput")
    d["entdb"] = nc.dram_tensor("entdb", [SLOT, NBLK * TD], BF16,
                                kind="ExternalInput")
    d["padc"] = nc.dram_tensor("padc", [SLOT, NBLK], F32, kind="ExternalInput")
    d["w1t"] = nc.dram_tensor("w1t", [TD, D], BF16, kind="ExternalInput")
    d["b1"] = nc.dram_tensor("b1", [1, D], BF16, kind="ExternalInput")
    d["w2t"] = nc.dram_tensor("w2t", [TD, D], BF16, kind="ExternalInput")
    d["b2"] = nc.dram_tensor("b2", [1, D], BF16, kind="ExternalInput")
    d["ex_o"] = nc.dram_tensor("ex_o", [SLOT, ECOLS], BF16,
                               kind="ExternalOutput")
    d["recden_o"] = nc.dram_tensor("recden_o", [SLOT, NBLK], F32,
                                   kind="ExternalOutput")
    d["ego_o"] = nc.dram_tensor("ego_o", [SLOT, NBLK * D], BF16,
                                kind="ExternalOutput")
    d["norm_o"] = nc.dram_tensor("norm_o", [SLOT, NBLK * D], F32,
                                 kind="ExternalOutput")

    with tile.TileContext(nc) as tc:
        with tc.tile_pool(name="pers", bufs=1) as pers:
            entdb_sb = pers.tile([SLOT, NBLK, TD], BF16)
            padc_sb = pers.tile([SLOT, NBLK], F32)
            ex_sb = pers.tile([SLOT, ECOLS], BF16)
            phs_sb = pers.tile([SLOT, ECOLS], BF16)
            mss_sb = pers.tile([SLOT, ECOLS], BF16)
            att_sb = pers.tile([SLOT, ECOLS], F32)
            den_sb = pers.tile([SLOT, NBLK], F32)
            recden_sb = pers.tile([SLOT, NBLK], F32)
            nh_sb = pers.tile([SLOT, NBLK, TD], BF16)
            o_sb = pers.tile([SLOT, NBLK, D], BF16)
            l1_sb = pers.tile([SLOT, NBLK, D], BF16)
            l2_sb = pers.tile([SLOT, NBLK, D], BF16)
            w1t_sb = pers.tile([TD, D], BF16)
            w2t_sb = pers.tile([TD, D], BF16)
            b1_sb = pers.tile([1, D], BF16)
            b2_sb = pers.tile([1, D], BF16)
            identb = pers.tile([SLOT, SLOT], BF16)
            ones_row = pers.tile([1, SLOT], BF16)

            nc.sync.dma_start(out=entdb_sb[:], in_=d["entdb"][:, :])
            nc.sync.dma_start(out=padc_sb[:], in_=d["padc"][:, :])
            nc.sync.dma_start(out=w1t_sb[:], in_=d["w1t"][:, :])
            nc.sync.dma_start(out=w2t_sb[:], in_=d["w2t"][:, :])
            nc.sync.dma_start(out=b1_sb[:], in_=d["b1"][:, :])
            nc.sync.dma_start(out=b2_sb[:], in_=d["b2"][:, :])
            make_identity(nc, identb[:])
            nc.vector.memset(ones_row[:], 1.0)
            nc.vector.memset(nh_sb[:], 0.0)
            nc.vector.memset(den_sb[:], 1.0)
            nc.vector.memset(ex_sb[:], 0.0)

            # -------- fused phase+mod score loop --------
            with tc.tile_pool(name="escore", bufs=2) as ep:
                po = mo = 0
                for (s0, gb, ls) in groups:
                    w = gb * ls * D
                    cols = slice(int(off[s0]), int(off[s0]) + gb * ls)
                    pht = ep.tile([SLOT, 2 * GMAX * D], BF16, tag="pht")
                    mdt = ep.tile([SLOT, 3 * GMAX * D], BF16, tag="mdt")
                    b = ep.tile([SLOT, GMAX * D], BF16, tag="b")
                    nc.sync.dma_start(out=pht[:, :2 * w],
                                      in_=d["ph_s"][:, po:po + 2 * w])
                    nc.sync.dma_start(out=mdt[:, :3 * w],
                                      in_=d["md_s"][:, mo:mo + 3 * w])
                    po += 2 * w
                    mo += 3 * w
                    ptf = pht[:, 0:w]
                    prf = pht[:, w:2 * w]
                    mtf = mdt[:, 0:w]
                    c1f = mdt[:, w:2 * w]
                    brf = mdt[:, 2 * w:3 * w]
                    bf = b[:, :w]
                    pr4 = prf.rearrange("p (b l dd) -> p b l dd", b=gb, dd=D)
                    c14 = c1f.rearrange("p (b l dd) -> p b l dd", b=gb, dd=D)
                    b4 = bf.rearrange("p (b l dd) -> p b l dd", b=gb, dd=D)
                    ph_b = entdb_sb[:, s0:s0 + gb, 0:D].unsqueeze(2) \
                        .to_broadcast([SLOT, gb, ls, D])
                    mh_b = entdb_sb[:, s0:s0 + gb, D:TD].unsqueeze(2) \
                        .to_broadcast([SLOT, gb, ls, D])
                    # phase: s1 = (pr - pt) + ph; sin; sum |.|  (no wrap:
                    # |arg| <= 1.5pi and the sin table stays close enough)
                    nc.vector.tensor_tensor(out=prf, in0=prf, in1=ptf,
                                            op=OP.subtract)
                    nc.vector.tensor_tensor(out=pr4, in0=pr4, in1=ph_b,
                                            op=OP.add)
                    nc.scalar.activation(out=bf, in_=prf, func=AF.Sin,
                                         scale=SIN_SCALE)
                    with nc.allow_low_precision(reason="bf16 att scores"):
                        nc.vector.tensor_reduce(
                            out=phs_sb[:, cols].rearrange("p (b l) -> p b l",
                                                          b=gb),
                            in_=b4, axis=mybir.AxisListType.X, op=OP.add,
                            apply_absolute_value=True)
                    # mod: v = mh*c1 - mt*c2 ; mss = sum v^2
                    nc.gpsimd.tensor_tensor(out=mtf, in0=mtf, in1=brf,
                                            op=OP.mult)
                    nc.vector.tensor_tensor(out=c14, in0=c14, in1=mh_b,
                                            op=OP.mult)
                    nc.vector.tensor_tensor(out=c1f, in0=c1f, in1=mtf,
                                            op=OP.subtract)
                    nc.scalar.activation(out=bf, in_=c1f, func=AF.Square)
                    with nc.allow_low_precision(reason="bf16 att scores"):
                        nc.vector.tensor_reduce(
                            out=mss_sb[:, cols].rearrange("p (b l) -> p b l",
                                                          b=gb),
                            in_=b4, axis=mybir.AxisListType.X, op=OP.add)

            # -------- att + softmax numerator (batched) --------
            nc.scalar.activation(out=mss_sb[:], in_=mss_sb[:], func=AF.Sqrt,
                                 scale=float(mod_w * mod_w))
            nc.vector.scalar_tensor_tensor(out=att_sb[:], in0=phs_sb[:],
                                           scalar=float(phase_w),
                                           in1=mss_sb[:], op0=OP.mult,
                                           op1=OP.add)
            nc.scalar.activation(out=ex_sb[:], in_=att_sb[:], func=AF.Exp)
            nc.vector.memset(recden_sb[:], 1.0)
            nc.sync.dma_start(out=d["ex_o"][:, :], in_=ex_sb[:])

            # -------- msg + Nh + dense, interleaved per group --------
            def dense_blk(s, dp, pp):
                x1tp = pp.tile([SLOT, SLOT], BF16, tag="x1tp")
                x2tp = pp.tile([SLOT, SLOT], BF16, tag="x2tp")
                nc.tensor.transpose(out=x1tp[:], in_=x1_sb[:, s, :],
                                    identity=identb[:])
                nc.tensor.transpose(out=x2tp[:], in_=x2_sb[:, s, :],
                                    identity=identb[:])
                x1t = dp.tile([SLOT, SLOT], BF16, tag="x1t")
                x2t = dp.tile([SLOT, SLOT], BF16, tag="x2t")
                nc.scalar.copy(out=x1t[:], in_=x1tp[:])
                nc.scalar.copy(out=x2t[:], in_=x2tp[:])
                o1p = pp.tile([SLOT, D], F32, tag="o1p")
                o2p = pp.tile([SLOT, D], F32, tag="o2p")
                nc.tensor.matmul(out=o1p[:], lhsT=x1t[:], rhs=w1t_sb[:],
                                 start=True, stop=False)
                nc.tensor.matmul(out=o1p[:], lhsT=ones_row[:], rhs=b1_sb[:],
                                 start=False, stop=True)
                nc.tensor.matmul(out=o2p[:], lhsT=x2t[:], rhs=w2t_sb[:],
                                 start=True, stop=False)
                nc.tensor.matmul(out=o2p[:], lhsT=ones_row[:], rhs=b2_sb[:],
                                 start=False, stop=True)
                nc.scalar.activation(out=l1_sb[:, s, :], in_=o1p[:],
                                     func=AF.Lrelu, alpha=0.01)
                nc.scalar.activation(out=l2_sb[:, s, :], in_=o2p[:],
                                     func=AF.Lrelu, alpha=0.01)

            with tc.tile_pool(name="xd", bufs=1) as xdp, \
                 tc.tile_pool(name="emsg", bufs=3) as mp, \
                 tc.tile_pool(name="dense", bufs=4) as dp, \
                 tc.tile_pool(name="dpsum", bufs=2, space="PSUM") as pp:
                x1_sb = xdp.tile([SLOT, NBLK, TD], BF16)
                x2_sb = xdp.tile([SLOT, NBLK, TD], BF16)
                so = 0
                cur = 0
                for (s0, gb, ls) in groups:
                    wm = gb * ls * TD
                    cols = slice(int(off[s0]), int(off[s0]) + gb * ls)
                    blks = slice(s0, s0 + gb)
                    nc.vector.tensor_reduce(
                        out=den_sb[:, blks],
                        in_=ex_sb[:, cols].rearrange("p (b l) -> p b l",
                                                     b=gb),
                        axis=mybir.AxisListType.X, op=OP.add)
                    nc.vector.tensor_tensor(out=den_sb[:, blks],
                                            in0=den_sb[:, blks],
                                            in1=padc_sb[:, blks],
                                            op=OP.subtract)
                    nc.vector.tensor_scalar_max(out=den_sb[:, blks],
                                                in0=den_sb[:, blks],
                                                scalar1=1e-30)
                    nc.vector.reciprocal(out=recden_sb[:, blks],
                                         in_=den_sb[:, blks])
                    ms = mp.tile([SLOT, GMAX * TD], BF16, tag="ms")
                    nc.sync.dma_start(out=ms[:, :wm],
                                      in_=d["ms_s"][:, so:so + wm])
                    so += wm
                    ms4 = ms[:, :wm].rearrange("p (b dd l) -> p b dd l",
                                               b=gb, dd=TD)
                    ex4 = ex_sb[:, cols].rearrange("p (b l) -> p b l", b=gb) \
                        .unsqueeze(2).to_broadcast([SLOT, gb, TD, ls])
                    nc.vector.tensor_tensor(out=ms4, in0=ms4, in1=ex4,
                                            op=OP.mult)
                    with nc.allow_low_precision(reason="bf16 Nh"):
                        nc.vector.tensor_reduce(out=nh_sb[:, blks, :],
                                                in_=ms4,
                                                axis=mybir.AxisListType.X,
                                                op=OP.add)
                    rdb = recden_sb[:, blks].unsqueeze(2).to_broadcast(
                        [SLOT, gb, TD])
                    nc.gpsimd.tensor_tensor(out=x2_sb[:, blks, :],
                                            in0=nh_sb[:, blks, :], in1=rdb,
                                            op=OP.mult)
                    nc.gpsimd.tensor_tensor(out=x1_sb[:, blks, :],
                                            in0=x2_sb[:, blks, :],
                                            in1=entdb_sb[:, blks, :],
                                            op=OP.add)
                    nc.gpsimd.tensor_tensor(out=x2_sb[:, blks, :],
                                            in0=x2_sb[:, blks, :],
                                            in1=entdb_sb[:, blks, :],
                                            op=OP.mult)
                    for s in range(cur, s0 + gb):
                        dense_blk(s, dp, pp)
                    cur = s0 + gb
                # tail: blocks not covered by any group (all-zero-degree)
                for s in range(cur, NBLK):
                    nc.vector.tensor_tensor(out=x2_sb[:, s, :],
                                            in0=nh_sb[:, s, :],
                                            in1=recden_sb[:, s:s + 1]
                                            .to_broadcast([SLOT, TD]),
                                            op=OP.mult)
                    nc.vector.tensor_tensor(out=x1_sb[:, s, :],
                                            in0=x2_sb[:, s, :],
                                            in1=entdb_sb[:, s, :], op=OP.add)
                    nc.vector.tensor_tensor(out=x2_sb[:, s, :],
                                            in0=x2_sb[:, s, :],
                                            in1=entdb_sb[:, s, :],
                                            op=OP.mult)
                    dense_blk(s, dp, pp)
            nc.sync.dma_start(out=d["recden_o"][:, :], in_=recden_sb[:])
            nc.vector.tensor_tensor(out=o_sb[:], in0=l1_sb[:], in1=l2_sb[:],
                                    op=OP.add)
            nc.sync.dma_start(
                out=d["ego_o"][:, :],
                in_=o_sb[:].rearrange("p b dd -> p (b dd)"))
            # -------- norm --------
            with tc.tile_pool(name="npool", bufs=1) as np_:
                sq_sb = np_.tile([SLOT, NBLK, D], BF16)
                nrm_sb = np_.tile([SLOT, NBLK, D], F32)
                nc.scalar.activation(out=sq_sb[:], in_=o_sb[:],
                                     func=AF.Square)
                ss = np_.tile([SLOT, NBLK], F32)
                nc.vector.tensor_reduce(out=ss[:], in_=sq_sb[:],
                                        axis=mybir.AxisListType.X, op=OP.add)
                nc.scalar.activation(out=ss[:], in_=ss[:], func=AF.Sqrt)
                nc.vector.tensor_scalar_max(out=ss[:], in0=ss[:],
                                            scalar1=1e-12)
                rs = np_.tile([SLOT, NBLK], F32)
                nc.vector.reciprocal(out=rs[:], in_=ss[:])
                nc.vector.tensor_tensor(
                    out=nrm_sb[:], in0=o_sb[:],
                    in1=rs[:].unsqueeze(2).to_broadcast([SLOT, NBLK, D]),
                    op=OP.mult)
                nc.sync.dma_start(
                    out=d["norm_o"][:, :],
                    in_=nrm_sb[:].rearrange("p b dd -> p (b dd)"))

    nc.compile()
    return nc


# ----------------------------------------------------------------- launch B/C
def build_BC(lay, din, dout):
    """Dense phase packs x1||x2 (and PK blocks) into one transpose+matmul
    with a block-diagonal weight tile."""
    L, off, ECOLS, groups = lay["L"], lay["off"], lay["ECOLS"], lay["groups"]
    NT = sum(gb * ls * din for (_, gb, ls) in groups)
    PK = SLOT // (2 * din)          # blocks per transpose (B:1, C:2)
    WC = 2 * PK * dout              # output cols per packed matmul
    nc = bacc.Bacc("TRN2", target_bir_lowering=False, debug=False,
                   num_devices=NCORES)
    d = {}
    d["t"] = nc.dram_tensor("t", [SLOT, NT], BF16, kind="ExternalInput")
    d["ex_i"] = nc.dram_tensor("ex_i", [SLOT, ECOLS], BF16,
                               kind="ExternalInput")
    d["recden_i"] = nc.dram_tensor("recden_i", [SLOT, NBLK], F32,
                                   kind="ExternalInput")
    d["egod"] = nc.dram_tensor("egod", [SLOT, NBLK * din], BF16,
                               kind="ExternalInput")
    d["wbd"] = nc.dram_tensor("wbd", [SLOT, WC], BF16, kind="ExternalInput")
    d["bbd"] = nc.dram_tensor("bbd", [1, WC], BF16, kind="ExternalInput")
    d["ego_o"] = nc.dram_tensor("ego_o", [SLOT, NBLK * dout], BF16,
                                kind="ExternalOutput")
    d["norm_o"] = nc.dram_tensor("norm_o", [SLOT, NBLK * dout], F32,
                                 kind="ExternalOutput")

    with tile.TileContext(nc) as tc:
        with tc.tile_pool(name="pers", bufs=1) as pers:
            ex_sb = pers.tile([SLOT, ECOLS], BF16)
            recden_sb = pers.tile([SLOT, NBLK], F32)
            egod_sb = pers.tile([SLOT, NBLK, din], BF16)
            nh_sb = pers.tile([SLOT, NBLK, din], BF16)
            xp_sb = pers.tile([SLOT, NBLK, 2, din], BF16)
            lp_sb = pers.tile([SLOT, NBLK, 2, dout], BF16)
            o_sb = pers.tile([SLOT, NBLK, dout], BF16)
            wbd_sb = pers.tile([SLOT, WC], BF16)
            bbd_sb = pers.tile([1, WC], BF16)
            identb = pers.tile([SLOT, SLOT], BF16)
            ones_row = pers.tile([1, SLOT], BF16)

            nc.sync.dma_start(out=ex_sb[:], in_=d["ex_i"][:, :])
            nc.sync.dma_start(out=recden_sb[:], in_=d["recden_i"][:, :])
            nc.sync.dma_start(out=egod_sb[:], in_=d["egod"][:, :])
            nc.sync.dma_start(out=wbd_sb[:], in_=d["wbd"][:, :])
            nc.sync.dma_start(out=bbd_sb[:], in_=d["bbd"][:, :])
            make_identity(nc, identb[:])
            nc.vector.memset(ones_row[:], 1.0)
            nc.vector.memset(nh_sb[:], 0.0)

            def dense_chunk(s, dp, pp):
                xtp = pp.tile([SLOT, SLOT], BF16, tag="xtp")
                nc.tensor.transpose(out=xtp[:], in_=xp_sb[:, s:s + PK, :, :],
                                    identity=identb[:])
                xt = dp.tile([SLOT, SLOT], BF16, tag="xt")
                nc.scalar.copy(out=xt[:], in_=xtp[:])
                op_ = pp.tile([SLOT, WC], F32, tag="op")
                nc.tensor.matmul(out=op_[:], lhsT=xt[:], rhs=wbd_sb[:],
                                 start=True, stop=False)
                nc.tensor.matmul(out=op_[:], lhsT=ones_row[:], rhs=bbd_sb[:],
                                 start=False, stop=True)
                nc.scalar.activation(out=lp_sb[:, s:s + PK, :, :],
                                     in_=op_[:], func=AF.Lrelu, alpha=0.01)

            def x_ops(blks, gb):
                rdb = recden_sb[:, blks].unsqueeze(2).to_broadcast(
                    [SLOT, gb, din])
                nhr = xp_sb[:, blks, 1, :]
                nc.gpsimd.tensor_tensor(out=nhr, in0=nh_sb[:, blks, :],
                                        in1=rdb, op=OP.mult)
                nc.gpsimd.tensor_tensor(out=xp_sb[:, blks, 0, :], in0=nhr,
                                        in1=egod_sb[:, blks, :], op=OP.add)
                nc.gpsimd.tensor_tensor(out=xp_sb[:, blks, 1, :], in0=nhr,
                                        in1=egod_sb[:, blks, :], op=OP.mult)

            with tc.tile_pool(name="emsg", bufs=3) as mp, \
                 tc.tile_pool(name="dense", bufs=4) as dp, \
                 tc.tile_pool(name="dpsum", bufs=2, space="PSUM") as pp:
                so = 0
                cur = 0
                for (s0, gb, ls) in groups:
                    wm = gb * ls * din
                    cols = slice(int(off[s0]), int(off[s0]) + gb * ls)
                    blks = slice(s0, s0 + gb)
                    ms = mp.tile([SLOT, GMAX * din], BF16, tag="ms")
                    nc.sync.dma_start(out=ms[:, :wm],
                                      in_=d["t"][:, so:so + wm])
                    so += wm
                    ms4 = ms[:, :wm].rearrange("p (b dd l) -> p b dd l",
                                               b=gb, dd=din)
                    ex4 = ex_sb[:, cols].rearrange("p (b l) -> p b l", b=gb) \
                        .unsqueeze(2).to_broadcast([SLOT, gb, din, ls])
                    nc.vector.tensor_tensor(out=ms4, in0=ms4, in1=ex4,
                                            op=OP.mult)
                    with nc.allow_low_precision(reason="bf16 Nh"):
                        nc.vector.tensor_reduce(out=nh_sb[:, blks, :],
                                                in_=ms4,
                                                axis=mybir.AxisListType.X,
                                                op=OP.add)
                    x_ops(blks, gb)
                    while cur + PK <= s0 + gb:
                        dense_chunk(cur, dp, pp)
                        cur += PK
                if cur < NBLK:
                    x_ops(slice(cur, NBLK), NBLK - cur)
                    while cur < NBLK:
                        dense_chunk(cur, dp, pp)
                        cur += PK
            nc.vector.tensor_tensor(out=o_sb[:], in0=lp_sb[:, :, 0, :],
                                    in1=lp_sb[:, :, 1, :], op=OP.add)
            nc.sync.dma_start(
                out=d["ego_o"][:, :],
                in_=o_sb[:].rearrange("p b dd -> p (b dd)"))
            with tc.tile_pool(name="npool", bufs=1) as np_:
                sq_sb = np_.tile([SLOT, NBLK, dout], BF16)
                nrm_sb = np_.tile([SLOT, NBLK, dout], F32)
                nc.scalar.activation(out=sq_sb[:], in_=o_sb[:],
                                     func=AF.Square)
                ss = np_.tile([SLOT, NBLK], F32)
                nc.vector.tensor_reduce(out=ss[:], in_=sq_sb[:],
                                        axis=mybir.AxisListType.X, op=OP.add)
                nc.scalar.activation(out=ss[:], in_=ss[:], func=AF.Sqrt)
                nc.vector.tensor_scalar_max(out=ss[:], in0=ss[:],
                                            scalar1=1e-12)
                rs = np_.tile([SLOT, NBLK], F32)
                nc.vector.reciprocal(out=rs[:], in_=ss[:])
                nc.vector.tensor_tensor(
                    out=nrm_sb[:], in0=o_sb[:],
                    in1=rs[:].unsqueeze(2).to_broadcast([SLOT, NBLK, dout]),
                    op=OP.mult)
                nc.sync.dma_start(
                    out=d["norm_o"][:, :],
                    in_=nrm_sb[:].rearrange("p b dd -> p (b dd)"))

    nc.compile()
    return nc


def make_wbd(W1, b1, W2, b2, din, dout):
    PK = SLOT // (2 * din)
    WC = 2 * PK * dout
    wbd = np.zeros((SLOT, WC), np.float32)
    bbd = np.zeros((1, WC), np.float32)
    for k in range(PK):
        r0 = k * 2 * din
        c0 = k * 2 * dout
        wbd[r0:r0 + din, c0:c0 + dout] = np.asarray(W1, np.float32).T
        wbd[r0 + din:r0 + 2 * din, c0 + dout:c0 + 2 * dout] = \
            np.asarray(W2, np.float32).T
        bbd[0, c0:c0 + dout] = np.asarray(b1, np.float32)
        bbd[0, c0 + dout:c0 + 2 * dout] = np.asarray(b2, np.float32)
    return wbd.astype(BF), bbd.astype(BF)


# ----------------------------------------------------------------- driver
def run(inp, trace=False, verbose=True):
    import time
    t0c = time.time()
    lay = host_prep(inp)
    if verbose:
        print(f"host_prep: ECOLS={lay['ECOLS']} groups={len(lay['groups'])} "
              f"({time.time()-t0c:.1f}s)")
    phase_w = float(np.asarray(inp["phase_w"]).reshape(-1)[0])
    mod_w = float(np.asarray(inp["mod_w"]).reshape(-1)[0])

    ph_l, md_l, ms_l, pc_l, eb_l = build_streams_A(lay)
    if verbose:
        print(f"streams built ({time.time()-t0c:.1f}s)")

    exec_ns = 0
    t0c = time.time()
    ncA = build_A(lay, phase_w, mod_w)
    if verbose:
        print(f"A compiled in {time.time()-t0c:.1f}s")
    in_maps = []
    for c in range(NCORES):
        in_maps.append(dict(
            ph_s=ph_l[c], md_s=md_l[c], ms_s=ms_l[c], entdb=eb_l[c],
            padc=pc_l[c],
            w1t=np.ascontiguousarray(np.asarray(inp["W1_0"]).T).astype(BF),
            b1=np.asarray(inp["b1_0"]).reshape(1, -1).astype(BF),
            w2t=np.ascontiguousarray(np.asarray(inp["W2_0"]).T).astype(BF),
            b2=np.asarray(inp["b2_0"]).reshape(1, -1).astype(BF),
        ))
    t0c = time.time()
    resA = run_bass_kernel_spmd(ncA, in_maps, core_ids=list(range(NCORES)),
                                trace=trace)
    if verbose:
        print(f"A ran in {time.time()-t0c:.1f}s exec_ns={resA.exec_time_ns}")
    if resA.exec_time_ns:
        exec_ns += resA.exec_time_ns

    ego1 = node_table(lay, [r["ego_o"] for r in resA.results], D)
    norm1 = node_table(lay, [r["norm_o"] for r in resA.results], D)
    ex_pc = [np.asarray(r["ex_o"]) for r in resA.results]
    recden_pc = [np.asarray(r["recden_o"]) for r in resA.results]

    t1s = build_tstream(lay, ego1, D)
    t0c = time.time()
    ncB = build_BC(lay, D, 32)
    if verbose:
        print(f"B compiled in {time.time()-t0c:.1f}s")
    wbdB, bbdB = make_wbd(inp["W1_1"], inp["b1_1"], inp["W2_1"],
                          inp["b2_1"], D, 32)
    in_maps = []
    for c in range(NCORES):
        in_maps.append(dict(
            t=t1s[c], ex_i=ex_pc[c], recden_i=recden_pc[c],
            egod=np.asarray(resA.results[c]["ego_o"]).astype(BF),
            wbd=wbdB, bbd=bbdB,
        ))
    t0c = time.time()
    resB = run_bass_kernel_spmd(ncB, in_maps, core_ids=list(range(NCORES)),
                                trace=trace)
    if verbose:
        print(f"B ran in {time.time()-t0c:.1f}s exec_ns={resB.exec_time_ns}")
    if resB.exec_time_ns:
        exec_ns += resB.exec_time_ns
    ego2 = node_table(lay, [r["ego_o"] for r in resB.results], 32)
    norm2 = node_table(lay, [r["norm_o"] for r in resB.results], 32)

    t2s = build_tstream(lay, ego2, 32)
    t0c = time.time()
    ncC = build_BC(lay, 32, 16)
    if verbose:
        print(f"C compiled in {time.time()-t0c:.1f}s")
    wbdC, bbdC = make_wbd(inp["W1_2"], inp["b1_2"], inp["W1_2b"],
                          inp["b2_2"], 32, 16)
    in_maps = []
    for c in range(NCORES):
        in_maps.append(dict(
            t=t2s[c], ex_i=ex_pc[c], recden_i=recden_pc[c],
            egod=np.asarray(resB.results[c]["ego_o"]).astype(BF),
            wbd=wbdC, bbd=bbdC,
        ))
    t0c = time.time()
    resC = run_bass_kernel_spmd(ncC, in_maps, core_ids=list(range(NCORES)),
                                trace=trace)
    if verbose:
        print(f"C ran in {time.time()-t0c:.1f}s exec_ns={resC.exec_time_ns}")
    if resC.exec_time_ns:
        exec_ns += resC.exec_time_ns
    norm3 = node_table(lay, [r["norm_o"] for r in resC.results], 16)

    ent = np.asarray(inp["entity_embed"], dtype=np.float32)
    out = np.concatenate([ent, norm1, norm2, norm3], axis=1)
    return out, exec_ns


# ----------------------------------------------------------------- entry
TRACE = False
LAST_EXEC_NS = None


def _install_ntff_hook():
    import sys, types
    if "antenv.axon_hooks" in sys.modules:
        return True
    try:
        mod = types.ModuleType("antenv.axon_hooks")
        mod._hook = None
        mod.set_axon_ntff_profile_hook = lambda h: setattr(mod, "_hook", h)
        mod.get_axon_ntff_profile_hook = lambda: mod._hook
        import antenv
        sys.modules["antenv.axon_hooks"] = mod
        antenv.axon_hooks = mod
        from trn_agent_boot.trn_boot import _ntff_profile_via_ctypes
        h = _ntff_profile_via_ctypes("/opt/axon/libaxon_pjrt.so")
        if h is None:
            return False
        mod._hook = h
        return True
    except Exception:
        return False


def kernel(**inputs):
    global LAST_EXEC_NS
    trace = TRACE and _install_ntff_hook()
    out, exec_ns = run(inputs, trace=trace, verbose=False)
    LAST_EXEC_NS = exec_ns
    return out
